# revision 9
# baseline (speedup 1.0000x reference)
"""Bigram (1-block transformer) LM forward pass on 8 TRN2 NeuronCores.

Sharding (8 cores, SPMD single NEFF):
  phase A: core c=(r=c//4, h=c%4) computes causal attention for batch row r, head h.
  exch 1 : 8-rank AllToAll redistributes attn from (row, head) to (row, quarter).
  phase B: core c=(r, q=c%4) computes x2=x+attn, LN2, FFN, residual for its
           512-token quarter; transposes result.
  exch 2 : 4-rank AllGather of xf^T within each row group.
  phase C: core c=(r, vq=c%4) computes logits[2048, 8000] for its vocab quarter,
           plus per-token sum(exp(logit)) partials for the CE loss.
Host assembles logits [2,2048,32000] and the scalar loss.

All matmuls run with float32r operands (TF32-like, ~1e-4 rel err, full PE rate).
"""

import sys
import os

for _p in ("/root/.axon_site", "/root/.axon_site/_ro/trn_rl_repo",
           "/root/.axon_site/_ro/pypackages", "/opt/trn_rl_repo"):
    if os.path.isdir(_p) and _p not in sys.path:
        sys.path.append(_p)

import numpy as np

import concourse.bass as bass
import concourse.mybir as mybir
import concourse.tile as tile
from concourse import bacc
from concourse.bass_utils import run_bass_kernel_spmd
from concourse.masks import make_identity

F32 = mybir.dt.float32
F32R = mybir.dt.float32r
I32 = mybir.dt.int32
AF = mybir.ActivationFunctionType
ALU = mybir.AluOpType
AXX = mybir.AxisListType.X

V, C, T, B, H, D = 32000, 1024, 2048, 2, 4, 256
P = 128
N_CORES = 8
TQ = T // 4           # 512 tokens per quarter
VS = V // 4           # 8000 vocab per core
NCH = VS // 16        # 500-wide lm chunks
KS = C // P           # 8 k-subtiles over C
F4 = 4 * C            # 4096 ffn hidden
EPS = 1e-5
SCALE = 1.0 / 32.0    # N_EMBED ** -0.5

DEBUG = False


def _r128(ap, pat):
    return ap.rearrange(pat, p=P)


def _bcast128(handle, n):
    """AP that replicates a [n] DRAM tensor across 128 partitions."""
    a = handle[:]
    return bass.AP(tensor=a.tensor, offset=0, ap=[[0, P]] + list(a.ap))


def build():
    nc = bacc.Bacc(None, target_bir_lowering=False, debug=False,
                   num_devices=N_CORES)

    # ---- I/O ----
    idx_row = nc.dram_tensor("idx_row", [T], I32, kind="ExternalInput")
    idx_q = nc.dram_tensor("idx_q", [TQ], I32, kind="ExternalInput")
    tok_emb = nc.dram_tensor("tok_emb", [V, C], F32, kind="ExternalInput")
    pos_emb = nc.dram_tensor("pos_emb", [T, C], F32, kind="ExternalInput")
    pos_q = nc.dram_tensor("pos_q", [TQ, C], F32, kind="ExternalInput")
    wq = nc.dram_tensor("wq", [C, D], F32R, kind="ExternalInput")
    wk = nc.dram_tensor("wk", [C, D], F32R, kind="ExternalInput")
    wv = nc.dram_tensor("wv", [C, D], F32R, kind="ExternalInput")
    ln1_g = nc.dram_tensor("ln1_g", [C], F32, kind="ExternalInput")
    ln1_b = nc.dram_tensor("ln1_b", [C], F32, kind="ExternalInput")
    ln2_g = nc.dram_tensor("ln2_g", [C], F32, kind="ExternalInput")
    ln2_b = nc.dram_tensor("ln2_b", [C], F32, kind="ExternalInput")
    w1 = nc.dram_tensor("w1", [C, F4], F32R, kind="ExternalInput")
    b1 = nc.dram_tensor("b1", [F4], F32, kind="ExternalInput")
    w2 = nc.dram_tensor("w2", [F4, C], F32R, kind="ExternalInput")
    b2 = nc.dram_tensor("b2", [C], F32, kind="ExternalInput")
    lm_ws = nc.dram_tensor("lm_ws", [C, VS], F32R, kind="ExternalInput")
    lm_bs = nc.dram_tensor("lm_bs", [VS], F32, kind="ExternalInput")
    rsel = nc.dram_tensor("rsel", [2], F32, kind="ExternalInput")

    logits_out = nc.dram_tensor("logits_out", [T, VS], F32, kind="ExternalOutput")
    sumexp_out = nc.dram_tensor("sumexp_out", [P, 16], F32, kind="ExternalOutput")
    if DEBUG:
        dbg_attn = nc.dram_tensor("dbg_attn", [16, P, D], F32, kind="ExternalOutput")
        dbg_x2 = nc.dram_tensor("dbg_x2", [P, 4, C], F32, kind="ExternalOutput")
        dbg_xf = nc.dram_tensor("dbg_xf", [P, 4, C], F32, kind="ExternalOutput")

    # collective bounce buffers
    cc_in = nc.dram_tensor("cc_in", [8, TQ, D], F32)          # a2a in
    cc_out = nc.dram_tensor("cc_out", [8, TQ, D], F32)        # a2a out
    ag2_in = nc.dram_tensor("ag2_in", [C, TQ], F32R)          # xfT chunk
    ag2_out = nc.dram_tensor("ag2_out", [4, C, TQ], F32R)

    G8 = [list(range(8))]
    G4 = [[0, 1, 2, 3], [4, 5, 6, 7]]

    with tile.TileContext(nc) as tc, \
         tc.tile_pool(name="const", bufs=1) as const:
        ident = const.tile([P, P], F32)
        make_identity(nc, ident[:])
        eps_t = const.tile([P, 1], F32)
        nc.vector.memset(eps_t[:], EPS)
        rsel_t = const.tile([P, 2], F32)
        nc.gpsimd.dma_start(rsel_t[:], _bcast128(rsel, 2))

        # 4 diagonal causal masks [128, 512] for m%4 = 0..3
        cmask = const.tile([P, 4, 512], F32)
        for k in range(4):
            nc.gpsimd.memset(cmask[:, k, :], 0.0)
            nc.gpsimd.affine_select(
                out=cmask[:, k, :], in_=cmask[:, k, :],
                compare_op=ALU.is_ge, fill=-1e9,
                base=P * k, pattern=[[-1, 512]], channel_multiplier=1)

        # ---------------- phase A ----------------
        with tc.tile_pool(name="paw", bufs=1) as paw, \
             tc.tile_pool(name="pabig", bufs=1) as pabig:
            g1_t = paw.tile([P, C], F32)
            b1g_t = paw.tile([P, C], F32)
            nc.gpsimd.dma_start(g1_t[:], _bcast128(ln1_g, C))
            nc.gpsimd.dma_start(b1g_t[:], _bcast128(ln1_b, C))
            wq_t = paw.tile([P, KS, D], F32R)
            wk_t = paw.tile([P, KS, D], F32R)
            wv_t = paw.tile([P, KS, D], F32R)
            nc.sync.dma_start(wq_t[:], _r128(wq[:], "(ks p) d -> p ks d"))
            nc.sync.dma_start(wk_t[:], _r128(wk[:], "(ks p) d -> p ks d"))
            nc.sync.dma_start(wv_t[:], _r128(wv[:], "(ks p) d -> p ks d"))
            idxr_t = paw.tile([P, 16], I32)
            nc.sync.dma_start(idxr_t[:], _r128(idx_row[:], "(m p) -> p m"))

            qT_t = pabig.tile([P, 2, T], F32R)
            kT_t = pabig.tile([P, 2, T], F32R)
            v_t = pabig.tile([P, 16, D], F32R)

            # ---- embed + LN1 + h^T + qkv, one 512-token chunk at a time
            with tc.tile_pool(name="pas", bufs=3) as pas, \
                 tc.tile_pool(name="pap", bufs=2, space="PSUM") as pap:
                for ch in range(4):
                    hT_t = pas.tile([P, KS, 512], F32R, tag="hT", bufs=2)
                    for j in range(4):
                        m = 4 * ch + j
                        x_t = pas.tile([P, C], F32, tag="x")
                        nc.sync.dma_start(x_t[:], pos_emb[P * m:P * (m + 1), :])
                        nc.gpsimd.indirect_dma_start(
                            out=x_t[:], out_offset=None, in_=tok_emb[:],
                            in_offset=bass.IndirectOffsetOnAxis(
                                ap=idxr_t[:, m:m + 1], axis=0),
                            compute_op=ALU.add)
                        # LN1
                        st = pas.tile([P, 2, 6], F32, tag="st")
                        nc.vector.bn_stats(out=st[:, 0, :], in_=x_t[:, 0:512])
                        nc.vector.bn_stats(out=st[:, 1, :], in_=x_t[:, 512:1024])
                        mv = pas.tile([P, 2], F32, tag="mv")
                        nc.vector.bn_aggr(out=mv[:], in_=st[:])
                        rstd = pas.tile([P, 1], F32, tag="rstd")
                        nc.scalar.activation(rstd[:], mv[:, 1:2], AF.Sqrt,
                                             bias=eps_t[:])
                        nc.vector.reciprocal(rstd[:], rstd[:])
                        h_t = pas.tile([P, C], F32, tag="h")
                        nc.vector.tensor_scalar(
                            out=h_t[:], in0=x_t[:], scalar1=mv[:, 0:1],
                            scalar2=rstd[:], op0=ALU.subtract, op1=ALU.mult)
                        nc.vector.tensor_mul(h_t[:], h_t[:], g1_t[:])
                        nc.vector.tensor_add(h_t[:], h_t[:], b1g_t[:])
                        # transpose h -> hT
                        for ks in range(KS):
                            ps = pap.tile([P, P], F32, tag="tp", bufs=2)
                            nc.tensor.transpose(ps[:], h_t[:, P * ks:P * (ks + 1)],
                                                ident[:])
                            nc.vector.tensor_copy(hT_t[:, ks, P * j:P * (j + 1)],
                                                  ps[:])
                    # qT / kT for this chunk  (moving = 512 tokens)
                    for dt_ in range(2):
                        psq = pap.tile([P, 512], F32, tag="ps512", bufs=3)
                        for k in range(KS):
                            nc.tensor.matmul(psq[:], wq_t[:, k, P * dt_:P * (dt_ + 1)],
                                             hT_t[:, k, :], start=(k == 0),
                                             stop=(k == KS - 1))
                        nc.vector.tensor_scalar_mul(
                            qT_t[:, dt_, 512 * ch:512 * (ch + 1)], psq[:], SCALE)
                        psk = pap.tile([P, 512], F32, tag="ps512", bufs=3)
                        for k in range(KS):
                            nc.tensor.matmul(psk[:], wk_t[:, k, P * dt_:P * (dt_ + 1)],
                                             hT_t[:, k, :], start=(k == 0),
                                             stop=(k == KS - 1))
                        nc.vector.tensor_copy(
                            kT_t[:, dt_, 512 * ch:512 * (ch + 1)], psk[:])
                    # v natural for this chunk (moving = wv 256)
                    for j in range(4):
                        psv = pap.tile([P, D], F32, tag="ps256", bufs=2)
                        for k in range(KS):
                            nc.tensor.matmul(psv[:], hT_t[:, k, P * j:P * (j + 1)],
                                             wv_t[:, k, :], start=(k == 0),
                                             stop=(k == KS - 1))
                        nc.vector.tensor_copy(v_t[:, 4 * ch + j, :], psv[:])

            # ---- causal attention, one 128-token q-tile at a time
            with tc.tile_pool(name="pat", bufs=3) as pat, \
                 tc.tile_pool(name="patp", bufs=2, space="PSUM") as patp:
                for m in range(16):
                    nch = m // 4 + 1
                    sc = pat.tile([P, T], F32, tag="sc", bufs=2)
                    for j in range(nch):
                        pss = patp.tile([P, 512], F32, tag="ps512", bufs=3)
                        for dt_ in range(2):
                            nc.tensor.matmul(
                                pss[:], qT_t[:, dt_, P * m:P * (m + 1)],
                                kT_t[:, dt_, 512 * j:512 * (j + 1)],
                                start=(dt_ == 0), stop=(dt_ == 1))
                        if j == m // 4:
                            nc.vector.tensor_add(sc[:, 512 * j:512 * (j + 1)],
                                                 pss[:], cmask[:, m % 4, :])
                        else:
                            nc.vector.tensor_copy(sc[:, 512 * j:512 * (j + 1)],
                                                  pss[:])
                    ngmx = pat.tile([P, 1], F32, tag="ngmx")
                    nc.vector.reduce_max(ngmx[:], sc[:, :512 * nch], axis=AXX,
                                         negate=True)
                    pp = pat.tile([P, T], F32, tag="pp", bufs=2)
                    sums = pat.tile([P, 4], F32, tag="sums")
                    for j in range(nch):
                        nc.scalar.activation(
                            pp[:, 512 * j:512 * (j + 1)],
                            sc[:, 512 * j:512 * (j + 1)], AF.Exp,
                            bias=ngmx[:], accum_out=sums[:, j:j + 1])
                    tot = pat.tile([P, 1], F32, tag="tot")
                    nc.vector.reduce_sum(tot[:], sums[:, :nch], axis=AXX)
                    nc.vector.reciprocal(tot[:], tot[:])
                    pT_t = pat.tile([P, 16, P], F32R, tag="pT", bufs=2)
                    for st_ in range(4 * nch):
                        ps = patp.tile([P, P], F32, tag="tp", bufs=2)
                        nc.tensor.transpose(ps[:], pp[:, P * st_:P * (st_ + 1)],
                                            ident[:])
                        nc.vector.tensor_copy(pT_t[:, st_, :], ps[:])
                    psav = patp.tile([P, D], F32, tag="ps256", bufs=2)
                    for st_ in range(4 * nch):
                        nc.tensor.matmul(psav[:], pT_t[:, st_, :], v_t[:, st_, :],
                                         start=(st_ == 0), stop=(st_ == 4 * nch - 1))
                    a0 = pat.tile([P, D], F32, tag="a0")
                    a1 = pat.tile([P, D], F32, tag="a1")
                    nc.vector.tensor_scalar(out=a0[:], in0=psav[:], scalar1=tot[:],
                                            scalar2=rsel_t[:, 0:1], op0=ALU.mult,
                                            op1=ALU.mult)
                    nc.vector.tensor_scalar(out=a1[:], in0=psav[:], scalar1=tot[:],
                                            scalar2=rsel_t[:, 1:2], op0=ALU.mult,
                                            op1=ALU.mult)
                    nc.sync.dma_start(
                        cc_in[0 * 4 + m // 4, P * (m % 4):P * (m % 4 + 1), :], a0[:])
                    nc.sync.dma_start(
                        cc_in[1 * 4 + m // 4, P * (m % 4):P * (m % 4 + 1), :], a1[:])
                    if DEBUG:
                        ad = pat.tile([P, D], F32, tag="ad")
                        nc.vector.tensor_scalar_mul(ad[:], psav[:], tot[:])
                        nc.sync.dma_start(dbg_attn[m, :, :], ad[:])

        # ---------------- exchange 1 ----------------
        nc.gpsimd.collective_compute(
            "AllToAll", ALU.bypass, replica_groups=G8,
            ins=[cc_in[:].opt()], outs=[cc_out[:].opt()])

        # ---------------- phase B ----------------
        with tc.tile_pool(name="pbw", bufs=1) as pbw:
            g2_t = pbw.tile([P, C], F32)
            b2g_t = pbw.tile([P, C], F32)
            nc.gpsimd.dma_start(g2_t[:], _bcast128(ln2_g, C))
            nc.gpsimd.dma_start(b2g_t[:], _bcast128(ln2_b, C))
            b1_sb = pbw.tile([P, F4 // P], F32)
            nc.sync.dma_start(b1_sb[:], _r128(b1[:], "(fo p) -> p fo"))
            b2_rep = pbw.tile([P, C], F32)
            nc.gpsimd.dma_start(b2_rep[:], _bcast128(b2, C))
            idxq_t = pbw.tile([P, 4], I32)
            nc.sync.dma_start(idxq_t[:], _r128(idx_q[:], "(m p) -> p m"))

            x2_t = pbw.tile([P, 4, C], F32)
            h2T_t = pbw.tile([P, KS, TQ], F32R)
            uT_t = pbw.tile([P, F4 // P, TQ], F32R)
            xf_t = pbw.tile([P, 4, C], F32)

            with tc.tile_pool(name="pbs", bufs=3) as pbs, \
                 tc.tile_pool(name="pbp1", bufs=2, space="PSUM") as pbp1:
                for j in range(4):
                    nc.sync.dma_start(x2_t[:, j, :], pos_q[P * j:P * (j + 1), :])
                    nc.gpsimd.indirect_dma_start(
                        out=x2_t[:, j, :], out_offset=None, in_=tok_emb[:],
                        in_offset=bass.IndirectOffsetOnAxis(
                            ap=idxq_t[:, j:j + 1], axis=0),
                        compute_op=ALU.add)
                    for hh in range(4):
                        ab = pbs.tile([P, D], F32, tag="ab")
                        nc.sync.dma_start(ab[:],
                                          cc_out[hh, P * j:P * (j + 1), :])
                        nc.vector.tensor_add(x2_t[:, j, D * hh:D * (hh + 1)],
                                             x2_t[:, j, D * hh:D * (hh + 1)], ab[:])
                        ab2 = pbs.tile([P, D], F32, tag="ab")
                        nc.sync.dma_start(ab2[:],
                                          cc_out[4 + hh, P * j:P * (j + 1), :])
                        nc.vector.tensor_add(x2_t[:, j, D * hh:D * (hh + 1)],
                                             x2_t[:, j, D * hh:D * (hh + 1)], ab2[:])
                    if DEBUG:
                        nc.sync.dma_start(dbg_x2[:, j, :], x2_t[:, j, :])
                    # LN2
                    st = pbs.tile([P, 2, 6], F32, tag="st")
                    nc.vector.bn_stats(out=st[:, 0, :], in_=x2_t[:, j, 0:512])
                    nc.vector.bn_stats(out=st[:, 1, :], in_=x2_t[:, j, 512:1024])
                    mv = pbs.tile([P, 2], F32, tag="mv")
                    nc.vector.bn_aggr(out=mv[:], in_=st[:])
                    rstd = pbs.tile([P, 1], F32, tag="rstd")
                    nc.scalar.activation(rstd[:], mv[:, 1:2], AF.Sqrt, bias=eps_t[:])
                    nc.vector.reciprocal(rstd[:], rstd[:])
                    h_t = pbs.tile([P, C], F32, tag="h")
                    nc.vector.tensor_scalar(
                        out=h_t[:], in0=x2_t[:, j, :], scalar1=mv[:, 0:1],
                        scalar2=rstd[:], op0=ALU.subtract, op1=ALU.mult)
                    nc.vector.tensor_mul(h_t[:], h_t[:], g2_t[:])
                    nc.vector.tensor_add(h_t[:], h_t[:], b2g_t[:])
                    for ks in range(KS):
                        ps = pbp1.tile([P, P], F32, tag="tp", bufs=2)
                        nc.tensor.transpose(ps[:], h_t[:, P * ks:P * (ks + 1)],
                                            ident[:])
                        nc.vector.tensor_copy(h2T_t[:, ks, P * j:P * (j + 1)], ps[:])

                # FFN1: uT[f, t] = relu(w1^T h2 + b1)
                for fc in range(8):
                    w1c = pbs.tile([P, KS, 512], F32R, tag="w1c", bufs=2)
                    nc.sync.dma_start(w1c[:],
                                      _r128(w1[:, 512 * fc:512 * (fc + 1)],
                                            "(ks p) f -> p ks f"))
                    for ft in range(4):
                        ftg = 4 * fc + ft
                        psu = pbp1.tile([P, 512], F32, tag="psu", bufs=3)
                        for k in range(KS):
                            nc.tensor.matmul(psu[:], w1c[:, k, P * ft:P * (ft + 1)],
                                             h2T_t[:, k, :], start=(k == 0),
                                             stop=(k == KS - 1))
                        nc.vector.tensor_scalar(
                            out=uT_t[:, ftg, :], in0=psu[:],
                            scalar1=b1_sb[:, ftg:ftg + 1], scalar2=0.0,
                            op0=ALU.add, op1=ALU.max)

            # FFN2 with all 8 psum banks accumulating over k
            with tc.tile_pool(name="pbs2", bufs=3) as pbs2, \
                 tc.tile_pool(name="pbp2", bufs=1, space="PSUM") as pbp2:
                psf = pbp2.tile([P, 8, 512], F32)
                for k2 in range(F4 // P):
                    w2s = pbs2.tile([P, C], F32R, tag="w2s")
                    nc.sync.dma_start(w2s[:], w2[P * k2:P * (k2 + 1), :])
                    for mtt in range(4):
                        for nn in range(2):
                            nc.tensor.matmul(
                                psf[:, 2 * mtt + nn, :],
                                uT_t[:, k2, P * mtt:P * (mtt + 1)],
                                w2s[:, 512 * nn:512 * (nn + 1)],
                                start=(k2 == 0), stop=(k2 == F4 // P - 1))
                for mtt in range(4):
                    for nn in range(2):
                        sl = slice(512 * nn, 512 * (nn + 1))
                        nc.vector.tensor_add(xf_t[:, mtt, sl],
                                             psf[:, 2 * mtt + nn, :], b2_rep[:, sl])
                        nc.vector.tensor_add(xf_t[:, mtt, sl], xf_t[:, mtt, sl],
                                             x2_t[:, mtt, sl])
                if DEBUG:
                    nc.sync.dma_start(dbg_xf[:], xf_t[:])

            # transpose xf -> xfT chunk and ship to allgather input
            with tc.tile_pool(name="pbs3", bufs=3) as pbs3, \
                 tc.tile_pool(name="pbp3", bufs=4, space="PSUM") as pbp3:
                xfT_t = pbs3.tile([P, KS, TQ], F32R, tag="xfT")
                for j in range(4):
                    for ks in range(KS):
                        ps = pbp3.tile([P, P], F32, tag="tp", bufs=2)
                        nc.tensor.transpose(ps[:], xf_t[:, j, P * ks:P * (ks + 1)],
                                            ident[:])
                        nc.vector.tensor_copy(xfT_t[:, ks, P * j:P * (j + 1)], ps[:])
                nc.sync.dma_start(_r128(ag2_in[:], "(ks p) t -> p ks t"), xfT_t[:])

        # ---------------- exchange 2 ----------------
        nc.gpsimd.collective_compute(
            "AllGather", ALU.bypass, replica_groups=G4,
            ins=[ag2_in[:].opt()], outs=[ag2_out[:].opt()])

        # ---------------- phase C: LM head + CE partials ----------------
        with tc.tile_pool(name="pcw", bufs=1) as pcw, \
             tc.tile_pool(name="pcs", bufs=4) as pcs, \
             tc.tile_pool(name="pcp", bufs=4, space="PSUM") as pcp:
            xfT_sb = pcw.tile([P, KS, 4, TQ], F32R)
            for qq in range(4):
                nc.sync.dma_start(xfT_sb[:, :, qq, :],
                                  _r128(ag2_out[qq], "(ks p) t -> p ks t"))
            se_acc = pcw.tile([P, 16], F32)
            nc.vector.memset(se_acc[:], 0.0)

            for n in range(16):
                wl = pcs.tile([P, KS, NCH], F32R, tag="wl", bufs=2)
                nc.sync.dma_start(wl[:], _r128(lm_ws[:, NCH * n:NCH * (n + 1)],
                                               "(ks p) v -> p ks v"))
                lmb = pcs.tile([P, NCH], F32, tag="lmb")
                a = lm_bs[:]
                nc.gpsimd.dma_start(
                    lmb[:], bass.AP(tensor=a.tensor, offset=NCH * n,
                                    ap=[[0, P], [1, NCH]]))
                for m in range(16):
                    psl = pcp.tile([P, NCH], F32, tag="psl", bufs=4)
                    for k in range(KS):
                        nc.tensor.matmul(
                            psl[:], xfT_sb[:, k, m // 4, P * (m % 4):P * (m % 4 + 1)],
                            wl[:, k, :], start=(k == 0), stop=(k == KS - 1))
                    lg = pcs.tile([P, NCH], F32, tag="lg")
                    nc.vector.tensor_add(lg[:], psl[:], lmb[:])
                    nc.sync.dma_start(
                        logits_out[P * m:P * (m + 1), NCH * n:NCH * (n + 1)], lg[:])
                    ex = pcs.tile([P, NCH], F32, tag="ex")
                    acc = pcs.tile([P, 1], F32, tag="acc")
                    nc.scalar.activation(ex[:], lg[:], AF.Exp, accum_out=acc[:])
                    nc.vector.tensor_add(se_acc[:, m:m + 1], se_acc[:, m:m + 1],
                                         acc[:])
            nc.sync.dma_start(sumexp_out[:], se_acc[:])

    nc.compile()
    return nc


_NC = None


def _get_nc():
    global _NC
    if _NC is None:
        _NC = build()
    return _NC


def _make_in_maps(inputs):
    f32 = np.float32
    idx = np.asarray(inputs["idx"], np.int32)
    tok_emb = np.ascontiguousarray(np.asarray(inputs["tok_emb"], f32))
    pos_emb = np.ascontiguousarray(np.asarray(inputs["pos_emb"], f32))
    w1 = np.ascontiguousarray(np.asarray(inputs["w1"], f32))
    b1 = np.ascontiguousarray(np.asarray(inputs["b1"], f32))
    w2 = np.ascontiguousarray(np.asarray(inputs["w2"], f32))
    b2 = np.ascontiguousarray(np.asarray(inputs["b2"], f32))
    lm_w = np.asarray(inputs["lm_w"], f32)
    lm_b = np.asarray(inputs["lm_b"], f32)
    wq, wk, wv = (np.asarray(inputs[k], f32) for k in ("wq", "wk", "wv"))
    maps = []
    for c in range(N_CORES):
        r, h = c // 4, c % 4
        q, vq = c % 4, c % 4
        maps.append({
            "idx_row": np.ascontiguousarray(idx[r]),
            "idx_q": np.ascontiguousarray(idx[r, TQ * q:TQ * (q + 1)]),
            "tok_emb": tok_emb,
            "pos_emb": pos_emb,
            "pos_q": np.ascontiguousarray(pos_emb[TQ * q:TQ * (q + 1)]),
            "wq": np.ascontiguousarray(wq[h]),
            "wk": np.ascontiguousarray(wk[h]),
            "wv": np.ascontiguousarray(wv[h]),
            "ln1_g": np.asarray(inputs["ln1_g"], f32),
            "ln1_b": np.asarray(inputs["ln1_b"], f32),
            "ln2_g": np.asarray(inputs["ln2_g"], f32),
            "ln2_b": np.asarray(inputs["ln2_b"], f32),
            "w1": w1, "b1": b1, "w2": w2, "b2": b2,
            "lm_ws": np.ascontiguousarray(lm_w[:, VS * vq:VS * (vq + 1)]),
            "lm_bs": np.ascontiguousarray(lm_b[VS * vq:VS * (vq + 1)]),
            "rsel": np.eye(2, dtype=f32)[r],
        })
    return maps


def _assemble(results, targets):
    logits = np.empty((B, T, V), np.float32)
    se = np.zeros((B, T), np.float64)
    for c in range(N_CORES):
        r, vq = c // 4, c % 4
        logits[r, :, VS * vq:VS * (vq + 1)] = results[c]["logits_out"]
        se[r] += results[c]["sumexp_out"].T.reshape(T).astype(np.float64)
    lse = np.log(se)
    tgt = np.asarray(targets)
    tl = np.take_along_axis(logits.astype(np.float64), tgt[..., None], axis=-1)[..., 0]
    loss = np.float32((lse - tl).mean())
    return logits, loss


def run(inputs, trace=False, trace_kwargs=None):
    nc = _get_nc()
    in_maps = _make_in_maps(inputs)
    res = run_bass_kernel_spmd(nc, in_maps, list(range(N_CORES)), trace=trace)
    logits, loss = _assemble(res.results, inputs["targets"])
    return logits, loss, res


def kernel(**inputs):
    logits, loss, _ = run(inputs, trace=False)
    return logits, loss


# revision 11
# speedup vs baseline: 1.2916x; 1.2916x over previous
"""Bigram (1-block transformer) LM forward pass on 8 TRN2 NeuronCores.

Sharding (8 cores, SPMD single NEFF):
  phase A: core c=(r=c//4, h=c%4) computes causal attention for batch row r, head h.
  exch 1 : 8-rank AllToAll redistributes attn from (row, head) to (row, quarter).
  phase B: core c=(r, q=c%4) computes x2=x+attn, LN2, FFN, residual for its
           512-token quarter; transposes result.
  exch 2 : 4-rank AllGather of xf^T (fp16) within each row group.
  phase C: core c=(r, vq=c%4) computes logits[2048, 8000] for its vocab quarter,
           plus per-token sum(exp(logit)) partials for the CE loss.
Host assembles logits [2,2048,32000] and the scalar loss.

Matmul operands are fp16 (weights pre-cast on host, activations cast on psum
eviction); accumulation is fp32.  LayerNorm gamma/beta are folded into the
following matmul's weights/bias on the host.  Attention computes transposed
score tiles exp(k^T q) so softmax needs no row max / no probability transpose;
the denominator comes from an appended ones-column on v.
"""

import sys
import os

for _p in ("/root/.axon_site", "/root/.axon_site/_ro/trn_rl_repo",
           "/root/.axon_site/_ro/pypackages", "/opt/trn_rl_repo"):
    if os.path.isdir(_p) and _p not in sys.path:
        sys.path.append(_p)

import numpy as np

import concourse.bass as bass
import concourse.mybir as mybir
import concourse.tile as tile
from concourse import bacc
from concourse.bass_utils import run_bass_kernel_spmd
from concourse.masks import make_identity

F32 = mybir.dt.float32
F16 = mybir.dt.float16
I32 = mybir.dt.int32
AF = mybir.ActivationFunctionType
ALU = mybir.AluOpType
AXX = mybir.AxisListType.X

V, C, T, B, H, D = 32000, 1024, 2048, 2, 4, 256
P = 128
N_CORES = 8
TQ = T // 4           # 512 tokens per quarter
VS = V // 4           # 8000 vocab per core
NCH = VS // 16        # 500-wide lm chunks
KS = C // P           # 8 k-subtiles over C
F4 = 4 * C            # 4096 ffn hidden
EPS = 1e-5

DEBUG = False


def _r128(ap, pat):
    return ap.rearrange(pat, p=P)


def _bcastap(handle, off, n):
    """AP replicating handle[off:off+n] across 128 partitions."""
    a = handle[:]
    return bass.AP(tensor=a.tensor, offset=off, ap=[[0, P], [1, n]])


def build():
    nc = bacc.Bacc(None, target_bir_lowering=False, debug=False,
                   num_devices=N_CORES)

    # ---- I/O ----
    idx_row = nc.dram_tensor("idx_row", [T], I32, kind="ExternalInput")
    idx_q = nc.dram_tensor("idx_q", [TQ], I32, kind="ExternalInput")
    tok_emb = nc.dram_tensor("tok_emb", [V, C], F32, kind="ExternalInput")
    pos_emb = nc.dram_tensor("pos_emb", [T, C], F32, kind="ExternalInput")
    pos_q = nc.dram_tensor("pos_q", [TQ, C], F32, kind="ExternalInput")
    wq = nc.dram_tensor("wq", [C, D], F16, kind="ExternalInput")   # g1-folded, /32
    wk = nc.dram_tensor("wk", [C, D], F16, kind="ExternalInput")   # g1-folded
    wv = nc.dram_tensor("wv", [C, D], F16, kind="ExternalInput")   # g1-folded
    qb = nc.dram_tensor("qb", [D], F32, kind="ExternalInput")
    kb = nc.dram_tensor("kb", [D], F32, kind="ExternalInput")
    vb = nc.dram_tensor("vb", [D], F32, kind="ExternalInput")
    w1 = nc.dram_tensor("w1", [C, F4], F16, kind="ExternalInput")  # g2-folded
    fb1 = nc.dram_tensor("fb1", [F4], F32, kind="ExternalInput")   # b1 + ln2_b@w1
    w2 = nc.dram_tensor("w2", [F4, C], F16, kind="ExternalInput")
    b2 = nc.dram_tensor("b2", [C], F32, kind="ExternalInput")
    lm_ws = nc.dram_tensor("lm_ws", [C, VS], F16, kind="ExternalInput")
    lm_bs = nc.dram_tensor("lm_bs", [VS], F32, kind="ExternalInput")
    rsel = nc.dram_tensor("rsel", [2], F32, kind="ExternalInput")

    logits_out = nc.dram_tensor("logits_out", [T, VS], F32, kind="ExternalOutput")
    sumexp_out = nc.dram_tensor("sumexp_out", [P, 16], F32, kind="ExternalOutput")
    if DEBUG:
        dbg_attn = nc.dram_tensor("dbg_attn", [16, P, D], F32, kind="ExternalOutput")
        dbg_x2 = nc.dram_tensor("dbg_x2", [P, 4, C], F32, kind="ExternalOutput")
        dbg_xf = nc.dram_tensor("dbg_xf", [P, 4, C], F32, kind="ExternalOutput")

    # collective bounce buffers (fp16 payloads)
    cc_in = nc.dram_tensor("cc_in", [8, TQ, D], F16)
    cc_out = nc.dram_tensor("cc_out", [8, TQ, D], F16)
    ag2_in = nc.dram_tensor("ag2_in", [C, TQ], F16)
    ag2_out = nc.dram_tensor("ag2_out", [4, C, TQ], F16)

    G8 = [list(range(8))]
    G4 = [[0, 1, 2, 3], [4, 5, 6, 7]]

    with tile.TileContext(nc) as tc, \
         tc.tile_pool(name="const", bufs=1) as const:
        ident = const.tile([P, P], F32)
        make_identity(nc, ident[:])
        eps_t = const.tile([P, 1], F32)
        nc.vector.memset(eps_t[:], EPS)
        rsel_t = const.tile([P, 2], F32)
        nc.gpsimd.dma_start(rsel_t[:], _bcastap(rsel, 0, 2))

        # ---------------- phase A ----------------
        with tc.tile_pool(name="paw", bufs=1) as paw, \
             tc.tile_pool(name="pabig", bufs=1) as pabig:
            wq_t = paw.tile([P, KS, D], F16)
            wk_t = paw.tile([P, KS, D], F16)
            wv_t = paw.tile([P, KS, D], F16)
            nc.sync.dma_start(wq_t[:], _r128(wq[:], "(ks p) d -> p ks d"))
            nc.sync.dma_start(wk_t[:], _r128(wk[:], "(ks p) d -> p ks d"))
            nc.sync.dma_start(wv_t[:], _r128(wv[:], "(ks p) d -> p ks d"))
            qb_t = paw.tile([P, 2], F32)
            kb_t = paw.tile([P, 2], F32)
            nc.sync.dma_start(qb_t[:], _r128(qb[:], "(dt p) -> p dt"))
            nc.sync.dma_start(kb_t[:], _r128(kb[:], "(dt p) -> p dt"))
            vb_t = paw.tile([P, D], F32)
            nc.gpsimd.dma_start(vb_t[:], _bcastap(vb, 0, D))
            idxr_t = paw.tile([P, 16], I32)
            nc.sync.dma_start(idxr_t[:], _r128(idx_row[:], "(m p) -> p m"))

            qT_t = pabig.tile([P, 2, T], F16)
            kT_t = pabig.tile([P, 2, T], F16)
            v_t = pabig.tile([P, 16, D + 1], F16)   # col D is the ones column
            nc.vector.memset(v_t[:], 1.0)

            # ---- embed + LN1 + h^T + qkv, one 512-token chunk at a time
            with tc.tile_pool(name="pas", bufs=3) as pas, \
                 tc.tile_pool(name="pap", bufs=2, space="PSUM") as pap:
                for ch in range(4):
                    hT_t = pas.tile([P, KS, 512], F16, tag="hT", bufs=2)
                    for j in range(4):
                        m = 4 * ch + j
                        x_t = pas.tile([P, C], F32, tag="x")
                        nc.sync.dma_start(x_t[:], pos_emb[P * m:P * (m + 1), :])
                        nc.gpsimd.indirect_dma_start(
                            out=x_t[:], out_offset=None, in_=tok_emb[:],
                            in_offset=bass.IndirectOffsetOnAxis(
                                ap=idxr_t[:, m:m + 1], axis=0),
                            compute_op=ALU.add)
                        # LN1 (gamma/beta folded into wq/wk/wv)
                        st = pas.tile([P, 2, 6], F32, tag="st")
                        nc.vector.bn_stats(out=st[:, 0, :], in_=x_t[:, 0:512])
                        nc.vector.bn_stats(out=st[:, 1, :], in_=x_t[:, 512:1024])
                        mv = pas.tile([P, 2], F32, tag="mv")
                        nc.vector.bn_aggr(out=mv[:], in_=st[:])
                        rstd = pas.tile([P, 1], F32, tag="rstd")
                        nc.scalar.activation(rstd[:], mv[:, 1:2], AF.Sqrt,
                                             bias=eps_t[:])
                        nc.vector.reciprocal(rstd[:], rstd[:])
                        h_t = pas.tile([P, C], F32, tag="h")
                        nc.vector.tensor_scalar(
                            out=h_t[:], in0=x_t[:], scalar1=mv[:, 0:1],
                            scalar2=rstd[:], op0=ALU.subtract, op1=ALU.mult)
                        # transpose h -> hT (batched 4-tile psum, then one cast)
                        for g in range(2):
                            ps = pap.tile([P, 512], F32, tag="tp512", bufs=2)
                            for i in range(4):
                                ks = 4 * g + i
                                nc.tensor.transpose(
                                    ps[:, P * i:P * (i + 1)],
                                    h_t[:, P * ks:P * (ks + 1)], ident[:])
                            nc.vector.tensor_copy(
                                hT_t[:, 4 * g:4 * (g + 1), P * j:P * (j + 1)],
                                ps[:].rearrange("p (i c) -> p i c", i=4))
                    # qT / kT for this chunk  (moving = 512 tokens)
                    for dt_ in range(2):
                        psq = pap.tile([P, 512], F32, tag="ps512", bufs=3)
                        for k in range(KS):
                            nc.tensor.matmul(psq[:], wq_t[:, k, P * dt_:P * (dt_ + 1)],
                                             hT_t[:, k, :], start=(k == 0),
                                             stop=(k == KS - 1))
                        nc.vector.tensor_scalar_add(
                            qT_t[:, dt_, 512 * ch:512 * (ch + 1)], psq[:],
                            qb_t[:, dt_:dt_ + 1])
                        psk = pap.tile([P, 512], F32, tag="ps512", bufs=3)
                        for k in range(KS):
                            nc.tensor.matmul(psk[:], wk_t[:, k, P * dt_:P * (dt_ + 1)],
                                             hT_t[:, k, :], start=(k == 0),
                                             stop=(k == KS - 1))
                        nc.vector.tensor_scalar_add(
                            kT_t[:, dt_, 512 * ch:512 * (ch + 1)], psk[:],
                            kb_t[:, dt_:dt_ + 1])
                    # v natural for this chunk (moving = wv 256)
                    for j in range(4):
                        psv = pap.tile([P, D], F32, tag="ps256", bufs=2)
                        for k in range(KS):
                            nc.tensor.matmul(psv[:], hT_t[:, k, P * j:P * (j + 1)],
                                             wv_t[:, k, :], start=(k == 0),
                                             stop=(k == KS - 1))
                        nc.vector.tensor_add(v_t[:, 4 * ch + j, :D], psv[:], vb_t[:])

            # ---- causal attention via transposed score tiles
            with tc.tile_pool(name="pat", bufs=2) as pat, \
                 tc.tile_pool(name="patp", bufs=2, space="PSUM") as patp:
                for tc_ in range(4):
                    n_st = 4 * tc_ + 4
                    expT = pat.tile([P, 16, 512], F16, tag="expT", bufs=2)
                    for st_ in range(n_st):
                        pss = patp.tile([P, 512], F32, tag="ps512", bufs=3)
                        for dt_ in range(2):
                            nc.tensor.matmul(
                                pss[:], kT_t[:, dt_, P * st_:P * (st_ + 1)],
                                qT_t[:, dt_, 512 * tc_:512 * (tc_ + 1)],
                                start=(dt_ == 0), stop=(dt_ == 1))
                        nc.scalar.activation(expT[:, st_, :], pss[:], AF.Exp)
                        if st_ >= 4 * tc_:
                            # zero entries with s > t on the diagonal blocks
                            nc.gpsimd.affine_select(
                                out=expT[:, st_, :], in_=expT[:, st_, :],
                                compare_op=ALU.is_ge, fill=0.0,
                                base=512 * tc_ - P * st_,
                                pattern=[[1, 512]], channel_multiplier=-1)
                    for tl in range(4):
                        m = 4 * tc_ + tl
                        psa = patp.tile([P, D + 1], F32, tag="psatt", bufs=4)
                        for st_ in range(n_st):
                            nc.tensor.matmul(psa[:], expT[:, st_, P * tl:P * (tl + 1)],
                                             v_t[:, st_, :], start=(st_ == 0),
                                             stop=(st_ == n_st - 1))
                        rcp = pat.tile([P, 1], F32, tag="rcp")
                        nc.vector.reciprocal(rcp[:], psa[:, D:D + 1])
                        a0 = pat.tile([P, D], F16, tag="a0")
                        a1 = pat.tile([P, D], F16, tag="a1")
                        nc.vector.tensor_scalar(out=a0[:], in0=psa[:, :D],
                                                scalar1=rcp[:],
                                                scalar2=rsel_t[:, 0:1],
                                                op0=ALU.mult, op1=ALU.mult)
                        nc.vector.tensor_scalar(out=a1[:], in0=psa[:, :D],
                                                scalar1=rcp[:],
                                                scalar2=rsel_t[:, 1:2],
                                                op0=ALU.mult, op1=ALU.mult)
                        nc.sync.dma_start(
                            cc_in[m // 4, P * (m % 4):P * (m % 4 + 1), :], a0[:])
                        nc.sync.dma_start(
                            cc_in[4 + m // 4, P * (m % 4):P * (m % 4 + 1), :], a1[:])
                        if DEBUG:
                            ad = pat.tile([P, D], F32, tag="ad")
                            nc.vector.tensor_scalar_mul(ad[:], psa[:, :D], rcp[:])
                            nc.sync.dma_start(dbg_attn[m, :, :], ad[:])

        # ---------------- exchange 1 ----------------
        nc.gpsimd.collective_compute(
            "AllToAll", ALU.bypass, replica_groups=G8,
            ins=[cc_in[:].opt()], outs=[cc_out[:].opt()])

        # ---------------- phase B ----------------
        with tc.tile_pool(name="pbw", bufs=1) as pbw:
            fb1_sb = pbw.tile([P, F4 // P], F32)
            nc.sync.dma_start(fb1_sb[:], _r128(fb1[:], "(fo p) -> p fo"))
            b2_rep = pbw.tile([P, C], F32)
            nc.gpsimd.dma_start(b2_rep[:], _bcastap(b2, 0, C))
            idxq_t = pbw.tile([P, 4], I32)
            nc.sync.dma_start(idxq_t[:], _r128(idx_q[:], "(m p) -> p m"))

            x2_t = pbw.tile([P, 4, C], F32)
            h2T_t = pbw.tile([P, KS, TQ], F16)
            uT_t = pbw.tile([P, F4 // P, TQ], F16)
            xf_t = pbw.tile([P, 4, C], F32)

            with tc.tile_pool(name="pbs", bufs=3) as pbs, \
                 tc.tile_pool(name="pbp1", bufs=2, space="PSUM") as pbp1:
                for j in range(4):
                    nc.sync.dma_start(x2_t[:, j, :], pos_q[P * j:P * (j + 1), :])
                    nc.gpsimd.indirect_dma_start(
                        out=x2_t[:, j, :], out_offset=None, in_=tok_emb[:],
                        in_offset=bass.IndirectOffsetOnAxis(
                            ap=idxq_t[:, j:j + 1], axis=0),
                        compute_op=ALU.add)
                    for hh in range(4):
                        ab = pbs.tile([P, D], F16, tag="ab")
                        nc.sync.dma_start(ab[:], cc_out[hh, P * j:P * (j + 1), :])
                        nc.vector.tensor_add(x2_t[:, j, D * hh:D * (hh + 1)],
                                             x2_t[:, j, D * hh:D * (hh + 1)], ab[:])
                        ab2 = pbs.tile([P, D], F16, tag="ab")
                        nc.sync.dma_start(ab2[:], cc_out[4 + hh, P * j:P * (j + 1), :])
                        nc.vector.tensor_add(x2_t[:, j, D * hh:D * (hh + 1)],
                                             x2_t[:, j, D * hh:D * (hh + 1)], ab2[:])
                    if DEBUG:
                        nc.sync.dma_start(dbg_x2[:, j, :], x2_t[:, j, :])
                    # LN2 (gamma/beta folded into w1/fb1)
                    st = pbs.tile([P, 2, 6], F32, tag="st")
                    nc.vector.bn_stats(out=st[:, 0, :], in_=x2_t[:, j, 0:512])
                    nc.vector.bn_stats(out=st[:, 1, :], in_=x2_t[:, j, 512:1024])
                    mv = pbs.tile([P, 2], F32, tag="mv")
                    nc.vector.bn_aggr(out=mv[:], in_=st[:])
                    rstd = pbs.tile([P, 1], F32, tag="rstd")
                    nc.scalar.activation(rstd[:], mv[:, 1:2], AF.Sqrt, bias=eps_t[:])
                    nc.vector.reciprocal(rstd[:], rstd[:])
                    h_t = pbs.tile([P, C], F32, tag="h")
                    nc.vector.tensor_scalar(
                        out=h_t[:], in0=x2_t[:, j, :], scalar1=mv[:, 0:1],
                        scalar2=rstd[:], op0=ALU.subtract, op1=ALU.mult)
                    for g in range(2):
                        ps = pbp1.tile([P, 512], F32, tag="tp512", bufs=2)
                        for i in range(4):
                            ks = 4 * g + i
                            nc.tensor.transpose(ps[:, P * i:P * (i + 1)],
                                                h_t[:, P * ks:P * (ks + 1)], ident[:])
                        nc.vector.tensor_copy(
                            h2T_t[:, 4 * g:4 * (g + 1), P * j:P * (j + 1)],
                            ps[:].rearrange("p (i c) -> p i c", i=4))

                # FFN1: uT[f, t] = relu(w1^T h2 + fb1)
                for fc in range(8):
                    w1c = pbs.tile([P, KS, 512], F16, tag="w1c", bufs=2)
                    nc.sync.dma_start(w1c[:],
                                      _r128(w1[:, 512 * fc:512 * (fc + 1)],
                                            "(ks p) f -> p ks f"))
                    for ft in range(4):
                        ftg = 4 * fc + ft
                        psu = pbp1.tile([P, 512], F32, tag="psu", bufs=3)
                        for k in range(KS):
                            nc.tensor.matmul(psu[:], w1c[:, k, P * ft:P * (ft + 1)],
                                             h2T_t[:, k, :], start=(k == 0),
                                             stop=(k == KS - 1))
                        nc.vector.tensor_scalar(
                            out=uT_t[:, ftg, :], in0=psu[:],
                            scalar1=fb1_sb[:, ftg:ftg + 1], scalar2=0.0,
                            op0=ALU.add, op1=ALU.max)

            # FFN2 with all 8 psum banks accumulating over k
            with tc.tile_pool(name="pbs2", bufs=3) as pbs2, \
                 tc.tile_pool(name="pbp2", bufs=1, space="PSUM") as pbp2:
                psf = pbp2.tile([P, 8, 512], F32)
                for k2 in range(F4 // P):
                    w2s = pbs2.tile([P, C], F16, tag="w2s", bufs=3)
                    nc.sync.dma_start(w2s[:], w2[P * k2:P * (k2 + 1), :])
                    for mtt in range(4):
                        for nn in range(2):
                            nc.tensor.matmul(
                                psf[:, 2 * mtt + nn, :],
                                uT_t[:, k2, P * mtt:P * (mtt + 1)],
                                w2s[:, 512 * nn:512 * (nn + 1)],
                                start=(k2 == 0), stop=(k2 == F4 // P - 1))
                for mtt in range(4):
                    for nn in range(2):
                        sl = slice(512 * nn, 512 * (nn + 1))
                        nc.vector.tensor_add(xf_t[:, mtt, sl],
                                             psf[:, 2 * mtt + nn, :], b2_rep[:, sl])
                        nc.vector.tensor_add(xf_t[:, mtt, sl], xf_t[:, mtt, sl],
                                             x2_t[:, mtt, sl])
                if DEBUG:
                    nc.sync.dma_start(dbg_xf[:], xf_t[:])

            # transpose xf -> xfT chunk (fp16) and ship to allgather input
            with tc.tile_pool(name="pbs3", bufs=3) as pbs3, \
                 tc.tile_pool(name="pbp3", bufs=2, space="PSUM") as pbp3:
                xfT_t = pbs3.tile([P, KS, TQ], F16, tag="xfT", bufs=1)
                for j in range(4):
                    for g in range(2):
                        ps = pbp3.tile([P, 512], F32, tag="tp512", bufs=2)
                        for i in range(4):
                            ks = 4 * g + i
                            nc.tensor.transpose(ps[:, P * i:P * (i + 1)],
                                                xf_t[:, j, P * ks:P * (ks + 1)],
                                                ident[:])
                        nc.vector.tensor_copy(
                            xfT_t[:, 4 * g:4 * (g + 1), P * j:P * (j + 1)],
                            ps[:].rearrange("p (i c) -> p i c", i=4))
                nc.sync.dma_start(_r128(ag2_in[:], "(ks p) t -> p ks t"), xfT_t[:])

        # ---------------- exchange 2 ----------------
        nc.gpsimd.collective_compute(
            "AllGather", ALU.bypass, replica_groups=G4,
            ins=[ag2_in[:].opt()], outs=[ag2_out[:].opt()])

        # ---------------- phase C: LM head + CE partials ----------------
        with tc.tile_pool(name="pcw", bufs=1) as pcw, \
             tc.tile_pool(name="pcs", bufs=4) as pcs, \
             tc.tile_pool(name="pcp", bufs=4, space="PSUM") as pcp:
            xq_tiles = []
            for qq in range(4):
                xq = pcw.tile([P, KS, TQ], F16, name=f"xq{qq}")
                nc.sync.dma_start(xq[:], _r128(ag2_out[qq], "(ks p) t -> p ks t"))
                xq_tiles.append(xq)
            se_acc = pcw.tile([P, 16], F32)
            nc.vector.memset(se_acc[:], 0.0)

            for n in range(16):
                wl = pcs.tile([P, KS, NCH], F16, tag="wl", bufs=2)
                nc.sync.dma_start(wl[:], _r128(lm_ws[:, NCH * n:NCH * (n + 1)],
                                               "(ks p) v -> p ks v"))
                lmb = pcs.tile([P, NCH], F32, tag="lmb", bufs=2)
                nc.gpsimd.dma_start(lmb[:], _bcastap(lm_bs, NCH * n, NCH))
                for m in range(16):
                    psl = pcp.tile([P, NCH], F32, tag="psl", bufs=4)
                    for k in range(KS):
                        nc.tensor.matmul(
                            psl[:], xq_tiles[m // 4][:, k, P * (m % 4):P * (m % 4 + 1)],
                            wl[:, k, :], start=(k == 0), stop=(k == KS - 1))
                    lg = pcs.tile([P, NCH], F32, tag="lg")
                    nc.vector.tensor_add(lg[:], psl[:], lmb[:])
                    nc.sync.dma_start(
                        logits_out[P * m:P * (m + 1), NCH * n:NCH * (n + 1)], lg[:])
                    ex = pcs.tile([P, NCH], F32, tag="ex", bufs=2)
                    acc = pcs.tile([P, 1], F32, tag="acc")
                    nc.scalar.activation(ex[:], lg[:], AF.Exp, accum_out=acc[:])
                    nc.vector.tensor_add(se_acc[:, m:m + 1], se_acc[:, m:m + 1],
                                         acc[:])
            nc.sync.dma_start(sumexp_out[:], se_acc[:])

    nc.compile()
    return nc


_NC = None


def _get_nc():
    global _NC
    if _NC is None:
        _NC = build()
    return _NC


def _make_in_maps(inputs):
    f32, f16 = np.float32, np.float16
    idx = np.asarray(inputs["idx"], np.int32)
    tok_emb = np.ascontiguousarray(np.asarray(inputs["tok_emb"], f32))
    pos_emb = np.ascontiguousarray(np.asarray(inputs["pos_emb"], f32))
    g1 = np.asarray(inputs["ln1_g"], f32)
    bg1 = np.asarray(inputs["ln1_b"], f32)
    g2 = np.asarray(inputs["ln2_g"], f32)
    bg2 = np.asarray(inputs["ln2_b"], f32)
    w1 = np.asarray(inputs["w1"], f32)
    b1 = np.asarray(inputs["b1"], f32)
    w2 = np.asarray(inputs["w2"], f32)
    b2 = np.asarray(inputs["b2"], f32)
    lm_w = np.asarray(inputs["lm_w"], f32)
    lm_b = np.asarray(inputs["lm_b"], f32)
    wq, wk, wv = (np.asarray(inputs[k], f32) for k in ("wq", "wk", "wv"))
    SC = 1.0 / 32.0

    w1f = np.ascontiguousarray((w1 * g2[:, None]).astype(f16))
    fb1 = np.ascontiguousarray(b1 + bg2 @ w1)
    w2f = np.ascontiguousarray(w2.astype(f16))

    maps = []
    for c in range(N_CORES):
        r, h = c // 4, c % 4
        q, vq = c % 4, c % 4
        maps.append({
            "idx_row": np.ascontiguousarray(idx[r]),
            "idx_q": np.ascontiguousarray(idx[r, TQ * q:TQ * (q + 1)]),
            "tok_emb": tok_emb,
            "pos_emb": pos_emb,
            "pos_q": np.ascontiguousarray(pos_emb[TQ * q:TQ * (q + 1)]),
            "wq": np.ascontiguousarray((wq[h] * g1[:, None] * SC).astype(f16)),
            "wk": np.ascontiguousarray((wk[h] * g1[:, None]).astype(f16)),
            "wv": np.ascontiguousarray((wv[h] * g1[:, None]).astype(f16)),
            "qb": np.ascontiguousarray((bg1 @ wq[h]) * SC),
            "kb": np.ascontiguousarray(bg1 @ wk[h]),
            "vb": np.ascontiguousarray(bg1 @ wv[h]),
            "w1": w1f, "fb1": fb1, "w2": w2f, "b2": b2,
            "lm_ws": np.ascontiguousarray(lm_w[:, VS * vq:VS * (vq + 1)].astype(f16)),
            "lm_bs": np.ascontiguousarray(lm_b[VS * vq:VS * (vq + 1)]),
            "rsel": np.eye(2, dtype=f32)[r],
        })
    return maps


def _assemble(results, targets):
    logits = np.empty((B, T, V), np.float32)
    se = np.zeros((B, T), np.float64)
    for c in range(N_CORES):
        r, vq = c // 4, c % 4
        logits[r, :, VS * vq:VS * (vq + 1)] = results[c]["logits_out"]
        se[r] += results[c]["sumexp_out"].T.reshape(T).astype(np.float64)
    lse = np.log(se)
    tgt = np.asarray(targets)
    tl = np.take_along_axis(logits.astype(np.float64), tgt[..., None], axis=-1)[..., 0]
    loss = np.float32((lse - tl).mean())
    return logits, loss


def run(inputs, trace=False):
    nc = _get_nc()
    in_maps = _make_in_maps(inputs)
    res = run_bass_kernel_spmd(nc, in_maps, list(range(N_CORES)), trace=trace)
    logits, loss = _assemble(res.results, inputs["targets"])
    return logits, loss, res


def kernel(**inputs):
    logits, loss, _ = run(inputs, trace=False)
    return logits, loss


# revision 17
# speedup vs baseline: 1.3212x; 1.0230x over previous
"""Bigram (1-block transformer) LM forward pass on 8 TRN2 NeuronCores.

Sharding (8 cores, SPMD single NEFF):
  phase A: core c=(r=c//4, h=c%4) computes causal attention for batch row r, head h.
  exch 1 : 8-rank AllToAll redistributes attn from (row, head) to (row, quarter).
  phase B: core c=(r, q=c%4) computes x2=x+attn, LN2, FFN, residual for its
           512-token quarter; transposes result.
  exch 2 : 4-rank AllGather of xf^T (fp16) within each row group.
  phase C: core c=(r, vq=c%4) computes logits[2048, 8000] for its vocab quarter,
           plus per-token sum(exp(logit)) partials for the CE loss.
Host assembles logits [2,2048,32000] and the scalar loss.

Matmul operands are fp16 (weights pre-cast on host, activations cast on psum
eviction); accumulation is fp32.  LayerNorm gamma/beta are folded into the
following matmul's weights/bias on the host.  Attention computes transposed
score tiles exp(k^T q) so softmax needs no row max / no probability transpose;
the denominator comes from an appended ones-column on v.
"""

import sys
import os

for _p in ("/root/.axon_site", "/root/.axon_site/_ro/trn_rl_repo",
           "/root/.axon_site/_ro/pypackages", "/opt/trn_rl_repo"):
    if os.path.isdir(_p) and _p not in sys.path:
        sys.path.append(_p)

import numpy as np

import concourse.bass as bass
import concourse.mybir as mybir
import concourse.tile as tile
from concourse import bacc
from concourse.bass_utils import run_bass_kernel_spmd
from concourse.masks import make_identity

F32 = mybir.dt.float32
F16 = mybir.dt.float16
I32 = mybir.dt.int32
AF = mybir.ActivationFunctionType
ALU = mybir.AluOpType
AXX = mybir.AxisListType.X

V, C, T, B, H, D = 32000, 1024, 2048, 2, 4, 256
P = 128
N_CORES = 8
TQ = T // 4           # 512 tokens per quarter
VS = V // 4           # 8000 vocab per core
NCH = VS // 16        # 500-wide lm chunks
KS = C // P           # 8 k-subtiles over C
F4 = 4 * C            # 4096 ffn hidden
EPS = 1e-5

DEBUG = False


def _r128(ap, pat):
    return ap.rearrange(pat, p=P)


def _bcastap(handle, off, n):
    """AP replicating handle[off:off+n] across 128 partitions."""
    a = handle[:]
    return bass.AP(tensor=a.tensor, offset=off, ap=[[0, P], [1, n]])


def build():
    nc = bacc.Bacc(None, target_bir_lowering=False, debug=False,
                   num_devices=N_CORES)

    # ---- I/O ----
    idx_row = nc.dram_tensor("idx_row", [T], I32, kind="ExternalInput")
    idx_q = nc.dram_tensor("idx_q", [TQ], I32, kind="ExternalInput")
    tok_emb = nc.dram_tensor("tok_emb", [V, C], F32, kind="ExternalInput")
    pos_emb = nc.dram_tensor("pos_emb", [T, C], F32, kind="ExternalInput")
    pos_q = nc.dram_tensor("pos_q", [TQ, C], F32, kind="ExternalInput")
    wq = nc.dram_tensor("wq", [C, D], F16, kind="ExternalInput")   # g1-folded, /32
    wk = nc.dram_tensor("wk", [C, D], F16, kind="ExternalInput")   # g1-folded
    wv = nc.dram_tensor("wv", [C, D], F16, kind="ExternalInput")   # g1-folded
    qb = nc.dram_tensor("qb", [D], F32, kind="ExternalInput")
    kb = nc.dram_tensor("kb", [D], F32, kind="ExternalInput")
    vb = nc.dram_tensor("vb", [D], F32, kind="ExternalInput")
    w1 = nc.dram_tensor("w1", [C, F4], F16, kind="ExternalInput")  # g2-folded
    fb1 = nc.dram_tensor("fb1", [F4], F32, kind="ExternalInput")   # b1 + ln2_b@w1
    w2 = nc.dram_tensor("w2", [F4, C], F16, kind="ExternalInput")
    b2 = nc.dram_tensor("b2", [C], F32, kind="ExternalInput")
    lm_ws = nc.dram_tensor("lm_ws", [C, VS], F16, kind="ExternalInput")
    lm_bs = nc.dram_tensor("lm_bs", [VS], F32, kind="ExternalInput")
    rsel = nc.dram_tensor("rsel", [2], F32, kind="ExternalInput")

    logits_out = nc.dram_tensor("logits_out", [T, VS], F32, kind="ExternalOutput")
    sumexp_out = nc.dram_tensor("sumexp_out", [P, 16], F32, kind="ExternalOutput")
    if DEBUG:
        dbg_attn = nc.dram_tensor("dbg_attn", [16, P, D], F32, kind="ExternalOutput")
        dbg_x2 = nc.dram_tensor("dbg_x2", [P, 4, C], F32, kind="ExternalOutput")
        dbg_xf = nc.dram_tensor("dbg_xf", [P, 4, C], F32, kind="ExternalOutput")

    # collective bounce buffers (fp16 payloads)
    cc_in = nc.dram_tensor("cc_in", [8, TQ, D], F16)
    cc_out = nc.dram_tensor("cc_out", [8, TQ, D], F16)
    ag2_in = nc.dram_tensor("ag2_in", [C, TQ], F16)
    ag2_out = nc.dram_tensor("ag2_out", [4, C, TQ], F16)

    G8 = [list(range(8))]
    G4 = [[0, 1, 2, 3], [4, 5, 6, 7]]

    with tile.TileContext(nc) as tc, \
         tc.tile_pool(name="const", bufs=1) as const:
        ident = const.tile([P, P], F32)
        make_identity(nc, ident[:])
        eps_t = const.tile([P, 1], F32)
        nc.vector.memset(eps_t[:], EPS)
        rsel_t = const.tile([P, 2], F32)
        nc.gpsimd.dma_start(rsel_t[:], _bcastap(rsel, 0, 2))

        # ---------------- phase A ----------------
        with tc.tile_pool(name="paw", bufs=1) as paw, \
             tc.tile_pool(name="pabig", bufs=1) as pabig:
            wq_t = paw.tile([P, KS, D], F16)
            wk_t = paw.tile([P, KS, D], F16)
            wv_t = paw.tile([P, KS, D], F16)
            nc.sync.dma_start(wq_t[:], _r128(wq[:], "(ks p) d -> p ks d"))
            nc.sync.dma_start(wk_t[:], _r128(wk[:], "(ks p) d -> p ks d"))
            nc.sync.dma_start(wv_t[:], _r128(wv[:], "(ks p) d -> p ks d"))
            qb_t = paw.tile([P, 2], F32)
            kb_t = paw.tile([P, 2], F32)
            nc.sync.dma_start(qb_t[:], _r128(qb[:], "(dt p) -> p dt"))
            nc.sync.dma_start(kb_t[:], _r128(kb[:], "(dt p) -> p dt"))
            vb_t = paw.tile([P, D], F32)
            nc.gpsimd.dma_start(vb_t[:], _bcastap(vb, 0, D))
            idxr_t = paw.tile([P, 16], I32)
            nc.sync.dma_start(idxr_t[:], _r128(idx_row[:], "(m p) -> p m"))

            # per-512-chunk tiles so attention can start before all qkv done
            qT_c, kT_c, v_c = [], [], []
            for ch in range(4):
                qT_c.append(pabig.tile([P, 2, 512], F16, name=f"qTc{ch}"))
                kT_c.append(pabig.tile([P, 2, 512], F16, name=f"kTc{ch}"))
                vt = pabig.tile([P, 4, D + 1], F16, name=f"vc{ch}")
                nc.vector.memset(vt[:], 1.0)   # col D is the ones column
                v_c.append(vt)

            # ---- embed + LN1 + h^T + qkv, one 512-token chunk at a time
            with tc.tile_pool(name="pas", bufs=3) as pas, \
                 tc.tile_pool(name="pap", bufs=2, space="PSUM") as pap:
                for ch in range(4):
                    hT_t = pas.tile([P, KS, 512], F16, tag="hT", bufs=2)
                    for j in range(4):
                        m = 4 * ch + j
                        x_t = pas.tile([P, C], F32, tag="x")
                        nc.sync.dma_start(x_t[:], pos_emb[P * m:P * (m + 1), :])
                        nc.gpsimd.indirect_dma_start(
                            out=x_t[:], out_offset=None, in_=tok_emb[:],
                            in_offset=bass.IndirectOffsetOnAxis(
                                ap=idxr_t[:, m:m + 1], axis=0),
                            compute_op=ALU.add)
                        # LN1 (gamma/beta folded into wq/wk/wv)
                        st = pas.tile([P, 2, 6], F32, tag="st")
                        nc.vector.bn_stats(out=st[:, 0, :], in_=x_t[:, 0:512])
                        nc.vector.bn_stats(out=st[:, 1, :], in_=x_t[:, 512:1024])
                        mv = pas.tile([P, 2], F32, tag="mv")
                        nc.vector.bn_aggr(out=mv[:], in_=st[:])
                        rstd = pas.tile([P, 1], F32, tag="rstd")
                        nc.scalar.activation(rstd[:], mv[:, 1:2], AF.Sqrt,
                                             bias=eps_t[:])
                        nc.vector.reciprocal(rstd[:], rstd[:])
                        h_t = pas.tile([P, C], F32, tag="h")
                        nc.vector.tensor_scalar(
                            out=h_t[:], in0=x_t[:], scalar1=mv[:, 0:1],
                            scalar2=rstd[:], op0=ALU.subtract, op1=ALU.mult)
                        # transpose h -> hT (batched 4-tile psum, then one cast)
                        for g in range(2):
                            ps = pap.tile([P, 512], F32, tag="tp512", bufs=2)
                            for i in range(4):
                                ks = 4 * g + i
                                nc.tensor.transpose(
                                    ps[:, P * i:P * (i + 1)],
                                    h_t[:, P * ks:P * (ks + 1)], ident[:])
                            nc.vector.tensor_copy(
                                hT_t[:, 4 * g:4 * (g + 1), P * j:P * (j + 1)],
                                ps[:].rearrange("p (i c) -> p i c", i=4))
                    # qT / kT for this chunk  (moving = 512 tokens)
                    for dt_ in range(2):
                        psq = pap.tile([P, 512], F32, tag="ps512", bufs=3)
                        for k in range(KS):
                            nc.tensor.matmul(psq[:], wq_t[:, k, P * dt_:P * (dt_ + 1)],
                                             hT_t[:, k, :], start=(k == 0),
                                             stop=(k == KS - 1))
                        nc.vector.tensor_scalar_add(
                            qT_c[ch][:, dt_, :], psq[:], qb_t[:, dt_:dt_ + 1])
                        psk = pap.tile([P, 512], F32, tag="ps512", bufs=3)
                        for k in range(KS):
                            nc.tensor.matmul(psk[:], wk_t[:, k, P * dt_:P * (dt_ + 1)],
                                             hT_t[:, k, :], start=(k == 0),
                                             stop=(k == KS - 1))
                        nc.vector.tensor_scalar_add(
                            kT_c[ch][:, dt_, :], psk[:], kb_t[:, dt_:dt_ + 1])
                    # v natural for this chunk (moving = wv 256)
                    for j in range(4):
                        psv = pap.tile([P, D], F32, tag="ps256", bufs=2)
                        for k in range(KS):
                            nc.tensor.matmul(psv[:], hT_t[:, k, P * j:P * (j + 1)],
                                             wv_t[:, k, :], start=(k == 0),
                                             stop=(k == KS - 1))
                        nc.vector.tensor_add(v_c[ch][:, j, :D], psv[:], vb_t[:])

            # ---- causal attention via transposed score tiles
            with tc.tile_pool(name="pat", bufs=2) as pat, \
                 tc.tile_pool(name="patp", bufs=2, space="PSUM") as patp:
                for tc_ in range(4):
                    n_st = 4 * tc_ + 4
                    expT = pat.tile([P, 16, 512], F16, tag="expT", bufs=2)
                    for st_ in range(n_st):
                        pss = patp.tile([P, 512], F32, tag="ps512", bufs=3)
                        for dt_ in range(2):
                            nc.tensor.matmul(
                                pss[:],
                                kT_c[st_ // 4][:, dt_, P * (st_ % 4):P * (st_ % 4 + 1)],
                                qT_c[tc_][:, dt_, :],
                                start=(dt_ == 0), stop=(dt_ == 1))
                        nc.scalar.activation(expT[:, st_, :], pss[:], AF.Exp)
                        if st_ >= 4 * tc_:
                            # zero entries with s > t on the diagonal blocks
                            nc.gpsimd.affine_select(
                                out=expT[:, st_, :], in_=expT[:, st_, :],
                                compare_op=ALU.is_ge, fill=0.0,
                                base=512 * tc_ - P * st_,
                                pattern=[[1, 512]], channel_multiplier=-1)
                    for tl in range(4):
                        m = 4 * tc_ + tl
                        psa = patp.tile([P, D + 1], F32, tag="psatt", bufs=4)
                        for st_ in range(n_st):
                            nc.tensor.matmul(psa[:], expT[:, st_, P * tl:P * (tl + 1)],
                                             v_c[st_ // 4][:, st_ % 4, :],
                                             start=(st_ == 0),
                                             stop=(st_ == n_st - 1))
                        rcp = pat.tile([P, 1], F32, tag="rcp")
                        nc.vector.reciprocal(rcp[:], psa[:, D:D + 1])
                        a0 = pat.tile([P, D], F16, tag="a0")
                        a1 = pat.tile([P, D], F16, tag="a1")
                        nc.vector.tensor_scalar(out=a0[:], in0=psa[:, :D],
                                                scalar1=rcp[:],
                                                scalar2=rsel_t[:, 0:1],
                                                op0=ALU.mult, op1=ALU.mult)
                        nc.vector.tensor_scalar(out=a1[:], in0=psa[:, :D],
                                                scalar1=rcp[:],
                                                scalar2=rsel_t[:, 1:2],
                                                op0=ALU.mult, op1=ALU.mult)
                        nc.sync.dma_start(
                            cc_in[m // 4, P * (m % 4):P * (m % 4 + 1), :], a0[:])
                        nc.sync.dma_start(
                            cc_in[4 + m // 4, P * (m % 4):P * (m % 4 + 1), :], a1[:])
                        if DEBUG:
                            ad = pat.tile([P, D], F32, tag="ad")
                            nc.vector.tensor_scalar_mul(ad[:], psa[:, :D], rcp[:])
                            nc.sync.dma_start(dbg_attn[m, :, :], ad[:])

        # ---------------- exchange 1 ----------------
        nc.gpsimd.collective_compute(
            "AllToAll", ALU.bypass, replica_groups=G8,
            ins=[cc_in[:].opt()], outs=[cc_out[:].opt()])

        # ---------------- phase B ----------------
        with tc.tile_pool(name="pbw", bufs=1) as pbw:
            fb1_sb = pbw.tile([P, F4 // P], F32)
            nc.sync.dma_start(fb1_sb[:], _r128(fb1[:], "(fo p) -> p fo"))
            b2_rep = pbw.tile([P, C], F32)
            nc.gpsimd.dma_start(b2_rep[:], _bcastap(b2, 0, C))
            idxq_t = pbw.tile([P, 4], I32)
            nc.sync.dma_start(idxq_t[:], _r128(idx_q[:], "(m p) -> p m"))

            x2_t = pbw.tile([P, 4, C], F32)
            h2T_h = [pbw.tile([P, KS, 256], F16, name=f"h2Th{i}") for i in range(2)]
            uT_t = pbw.tile([P, F4 // P, TQ], F16)
            xf_t = pbw.tile([P, 4, C], F32)

            with tc.tile_pool(name="pbs", bufs=3) as pbs, \
                 tc.tile_pool(name="pbp1", bufs=2, space="PSUM") as pbp1:
                for j in range(4):
                    nc.sync.dma_start(x2_t[:, j, :], pos_q[P * j:P * (j + 1), :])
                    nc.gpsimd.indirect_dma_start(
                        out=x2_t[:, j, :], out_offset=None, in_=tok_emb[:],
                        in_offset=bass.IndirectOffsetOnAxis(
                            ap=idxq_t[:, j:j + 1], axis=0),
                        compute_op=ALU.add)
                    for hh in range(4):
                        ab = pbs.tile([P, D], F16, tag="ab")
                        nc.sync.dma_start(ab[:], cc_out[hh, P * j:P * (j + 1), :])
                        nc.vector.tensor_add(x2_t[:, j, D * hh:D * (hh + 1)],
                                             x2_t[:, j, D * hh:D * (hh + 1)], ab[:])
                        ab2 = pbs.tile([P, D], F16, tag="ab")
                        nc.sync.dma_start(ab2[:], cc_out[4 + hh, P * j:P * (j + 1), :])
                        nc.vector.tensor_add(x2_t[:, j, D * hh:D * (hh + 1)],
                                             x2_t[:, j, D * hh:D * (hh + 1)], ab2[:])
                    if DEBUG:
                        nc.sync.dma_start(dbg_x2[:, j, :], x2_t[:, j, :])
                    # LN2 (gamma/beta folded into w1/fb1)
                    st = pbs.tile([P, 2, 6], F32, tag="st")
                    nc.vector.bn_stats(out=st[:, 0, :], in_=x2_t[:, j, 0:512])
                    nc.vector.bn_stats(out=st[:, 1, :], in_=x2_t[:, j, 512:1024])
                    mv = pbs.tile([P, 2], F32, tag="mv")
                    nc.vector.bn_aggr(out=mv[:], in_=st[:])
                    rstd = pbs.tile([P, 1], F32, tag="rstd")
                    nc.scalar.activation(rstd[:], mv[:, 1:2], AF.Sqrt, bias=eps_t[:])
                    nc.vector.reciprocal(rstd[:], rstd[:])
                    h_t = pbs.tile([P, C], F32, tag="h")
                    nc.vector.tensor_scalar(
                        out=h_t[:], in0=x2_t[:, j, :], scalar1=mv[:, 0:1],
                        scalar2=rstd[:], op0=ALU.subtract, op1=ALU.mult)
                    for g in range(2):
                        ps = pbp1.tile([P, 512], F32, tag="tp512", bufs=2)
                        for i in range(4):
                            ks = 4 * g + i
                            nc.tensor.transpose(ps[:, P * i:P * (i + 1)],
                                                h_t[:, P * ks:P * (ks + 1)], ident[:])
                        nc.vector.tensor_copy(
                            h2T_h[j // 2][:, 4 * g:4 * (g + 1),
                                          P * (j % 2):P * (j % 2 + 1)],
                            ps[:].rearrange("p (i c) -> p i c", i=4))

                # FFN1: uT[f, t] = relu(w1^T h2 + fb1), token-halves
                for fc in range(8):
                    w1c = pbs.tile([P, KS, 512], F16, tag="w1c", bufs=2)
                    nc.sync.dma_start(w1c[:],
                                      _r128(w1[:, 512 * fc:512 * (fc + 1)],
                                            "(ks p) f -> p ks f"))
                    for ft in range(4):
                        ftg = 4 * fc + ft
                        for hf in range(2):
                            psu = pbp1.tile([P, 256], F32, tag="psu", bufs=3)
                            for k in range(KS):
                                nc.tensor.matmul(psu[:],
                                                 w1c[:, k, P * ft:P * (ft + 1)],
                                                 h2T_h[hf][:, k, :],
                                                 start=(k == 0), stop=(k == KS - 1))
                            nc.vector.tensor_scalar(
                                out=uT_t[:, ftg, 256 * hf:256 * (hf + 1)],
                                in0=psu[:],
                                scalar1=fb1_sb[:, ftg:ftg + 1], scalar2=0.0,
                                op0=ALU.add, op1=ALU.max)

            # FFN2 with all 8 psum banks accumulating over k
            with tc.tile_pool(name="pbs2", bufs=3) as pbs2, \
                 tc.tile_pool(name="pbp2", bufs=1, space="PSUM") as pbp2:
                psf = pbp2.tile([P, 8, 512], F32)
                for k2 in range(F4 // P):
                    w2s = pbs2.tile([P, C], F16, tag="w2s", bufs=3)
                    nc.sync.dma_start(w2s[:], w2[P * k2:P * (k2 + 1), :])
                    for mtt in range(4):
                        for nn in range(2):
                            nc.tensor.matmul(
                                psf[:, 2 * mtt + nn, :],
                                uT_t[:, k2, P * mtt:P * (mtt + 1)],
                                w2s[:, 512 * nn:512 * (nn + 1)],
                                start=(k2 == 0), stop=(k2 == F4 // P - 1))
                for mtt in range(4):
                    for nn in range(2):
                        sl = slice(512 * nn, 512 * (nn + 1))
                        nc.vector.tensor_add(xf_t[:, mtt, sl],
                                             psf[:, 2 * mtt + nn, :], b2_rep[:, sl])
                        nc.vector.tensor_add(xf_t[:, mtt, sl], xf_t[:, mtt, sl],
                                             x2_t[:, mtt, sl])
                if DEBUG:
                    nc.sync.dma_start(dbg_xf[:], xf_t[:])

            # transpose xf -> xfT chunk (fp16) and ship to allgather input
            with tc.tile_pool(name="pbs3", bufs=3) as pbs3, \
                 tc.tile_pool(name="pbp3", bufs=2, space="PSUM") as pbp3:
                xfT_t = pbs3.tile([P, KS, TQ], F16, tag="xfT", bufs=1)
                for j in range(4):
                    for g in range(2):
                        ps = pbp3.tile([P, 512], F32, tag="tp512", bufs=2)
                        for i in range(4):
                            ks = 4 * g + i
                            nc.tensor.transpose(ps[:, P * i:P * (i + 1)],
                                                xf_t[:, j, P * ks:P * (ks + 1)],
                                                ident[:])
                        nc.vector.tensor_copy(
                            xfT_t[:, 4 * g:4 * (g + 1), P * j:P * (j + 1)],
                            ps[:].rearrange("p (i c) -> p i c", i=4))
                nc.sync.dma_start(_r128(ag2_in[:], "(ks p) t -> p ks t"), xfT_t[:])

        # ---------------- exchange 2 ----------------
        nc.gpsimd.collective_compute(
            "AllGather", ALU.bypass, replica_groups=G4,
            ins=[ag2_in[:].opt()], outs=[ag2_out[:].opt()])

        # ---------------- phase C: LM head + CE partials ----------------
        with tc.tile_pool(name="pcw", bufs=1) as pcw, \
             tc.tile_pool(name="pcs", bufs=4) as pcs, \
             tc.tile_pool(name="pcp", bufs=4, space="PSUM") as pcp:
            xq_tiles = []
            for qq in range(4):
                xq = pcw.tile([P, KS, TQ], F16, name=f"xq{qq}")
                nc.sync.dma_start(xq[:], _r128(ag2_out[qq], "(ks p) t -> p ks t"))
                xq_tiles.append(xq)
            se_acc = pcw.tile([P, 16], F32)
            nc.vector.memset(se_acc[:], 0.0)

            for n in range(16):
                wl = pcs.tile([P, KS, NCH], F16, tag="wl", bufs=2)
                nc.sync.dma_start(wl[:], _r128(lm_ws[:, NCH * n:NCH * (n + 1)],
                                               "(ks p) v -> p ks v"))
                lmb = pcs.tile([P, NCH], F32, tag="lmb", bufs=2)
                nc.gpsimd.dma_start(lmb[:], _bcastap(lm_bs, NCH * n, NCH))
                for m in range(16):
                    psl = pcp.tile([P, NCH], F32, tag="psl", bufs=4)
                    for k in range(KS):
                        nc.tensor.matmul(
                            psl[:], xq_tiles[m // 4][:, k, P * (m % 4):P * (m % 4 + 1)],
                            wl[:, k, :], start=(k == 0), stop=(k == KS - 1))
                    lg = pcs.tile([P, NCH], F32, tag="lg")
                    nc.vector.tensor_add(lg[:], psl[:], lmb[:])
                    nc.sync.dma_start(
                        logits_out[P * m:P * (m + 1), NCH * n:NCH * (n + 1)], lg[:])
                    ex = pcs.tile([P, NCH], F32, tag="ex", bufs=2)
                    acc = pcs.tile([P, 1], F32, tag="acc")
                    nc.scalar.activation(ex[:], lg[:], AF.Exp, accum_out=acc[:])
                    nc.vector.tensor_add(se_acc[:, m:m + 1], se_acc[:, m:m + 1],
                                         acc[:])
            nc.sync.dma_start(sumexp_out[:], se_acc[:])

    nc.compile()
    return nc


_NC = None


def _get_nc():
    global _NC
    if _NC is None:
        _NC = build()
    return _NC


def _make_in_maps(inputs):
    f32, f16 = np.float32, np.float16
    idx = np.asarray(inputs["idx"], np.int32)
    tok_emb = np.ascontiguousarray(np.asarray(inputs["tok_emb"], f32))
    pos_emb = np.ascontiguousarray(np.asarray(inputs["pos_emb"], f32))
    g1 = np.asarray(inputs["ln1_g"], f32)
    bg1 = np.asarray(inputs["ln1_b"], f32)
    g2 = np.asarray(inputs["ln2_g"], f32)
    bg2 = np.asarray(inputs["ln2_b"], f32)
    w1 = np.asarray(inputs["w1"], f32)
    b1 = np.asarray(inputs["b1"], f32)
    w2 = np.asarray(inputs["w2"], f32)
    b2 = np.asarray(inputs["b2"], f32)
    lm_w = np.asarray(inputs["lm_w"], f32)
    lm_b = np.asarray(inputs["lm_b"], f32)
    wq, wk, wv = (np.asarray(inputs[k], f32) for k in ("wq", "wk", "wv"))
    SC = 1.0 / 32.0

    w1f = np.ascontiguousarray((w1 * g2[:, None]).astype(f16))
    fb1 = np.ascontiguousarray(b1 + bg2 @ w1)
    w2f = np.ascontiguousarray(w2.astype(f16))

    maps = []
    for c in range(N_CORES):
        r, h = c // 4, c % 4
        q, vq = c % 4, c % 4
        maps.append({
            "idx_row": np.ascontiguousarray(idx[r]),
            "idx_q": np.ascontiguousarray(idx[r, TQ * q:TQ * (q + 1)]),
            "tok_emb": tok_emb,
            "pos_emb": pos_emb,
            "pos_q": np.ascontiguousarray(pos_emb[TQ * q:TQ * (q + 1)]),
            "wq": np.ascontiguousarray((wq[h] * g1[:, None] * SC).astype(f16)),
            "wk": np.ascontiguousarray((wk[h] * g1[:, None]).astype(f16)),
            "wv": np.ascontiguousarray((wv[h] * g1[:, None]).astype(f16)),
            "qb": np.ascontiguousarray((bg1 @ wq[h]) * SC),
            "kb": np.ascontiguousarray(bg1 @ wk[h]),
            "vb": np.ascontiguousarray(bg1 @ wv[h]),
            "w1": w1f, "fb1": fb1, "w2": w2f, "b2": b2,
            "lm_ws": np.ascontiguousarray(lm_w[:, VS * vq:VS * (vq + 1)].astype(f16)),
            "lm_bs": np.ascontiguousarray(lm_b[VS * vq:VS * (vq + 1)]),
            "rsel": np.eye(2, dtype=f32)[r],
        })
    return maps


def _assemble(results, targets):
    logits = np.empty((B, T, V), np.float32)
    se = np.zeros((B, T), np.float64)
    for c in range(N_CORES):
        r, vq = c // 4, c % 4
        logits[r, :, VS * vq:VS * (vq + 1)] = results[c]["logits_out"]
        se[r] += results[c]["sumexp_out"].T.reshape(T).astype(np.float64)
    lse = np.log(se)
    tgt = np.asarray(targets)
    tl = np.take_along_axis(logits.astype(np.float64), tgt[..., None], axis=-1)[..., 0]
    loss = np.float32((lse - tl).mean())
    return logits, loss


def run(inputs, trace=False):
    nc = _get_nc()
    in_maps = _make_in_maps(inputs)
    res = run_bass_kernel_spmd(nc, in_maps, list(range(N_CORES)), trace=trace)
    logits, loss = _assemble(res.results, inputs["targets"])
    return logits, loss, res


def kernel(**inputs):
    logits, loss, _ = run(inputs, trace=False)
    return logits, loss
